# revision 15
# baseline (speedup 1.0000x reference)
"""Trainium2 Bass kernel for nn_AFSLSTM (LayerNorm -> sigmoid feature gate ->
bidirectional 1-step LSTM -> tiny MLP head).

Sharding: data-parallel over the batch dim, 1024 rows per core, weights
replicated. No collectives; host concatenates per-core outputs.

Device-side math (per core, feature-major layout [feature_part, batch_free]):
the gate and LSTM matmuls (97% of PE work) run as fp8-e4m3 DoubleRow matmuls
(two 128-row contraction chunks per instruction, ~1.4-1.8x bf16 rate), with
weights pre-scaled by 64 on the host so the uniform(-1/32,1/32) entries sit
in fp8's normal range. The 1/64 descale folds into the LN rsqrt (gate) and
the activation `scale` operand (LSTM). x ships twice: bf16 for LN stats and
the xg product, fp8 as the gate moving operand; xg itself is written as fp8
by the DVE so the LSTM matmuls consume it directly. Head stays bf16.
  G0 = Q8(64*Wg*ln_g).Q8(x)               (8 DoubleRow matmuls per j-chunk)
  gate = sigmoid((rsq/64)*(G0 + (-mu)(x)c1q) + bg)   (c1q = colsums of the
        quantized weights, so the mu-correction matches the fp8 matmul)
  xg = fp8(x * gate)
  pre_{i,g,o} = Q8(64*W_ih[{i,g,o}]).xg    (f-gate dropped: c0 = 0)
  h = sigmoid(pre_o/64 + b_o) * tanh(sigmoid(pre_i/64 + b_i) * tanh(pre_g/64 + b_g))
  hid = relu(W1.feat + b1);  out = W2.hid + b2
LN statistics are folded to [128, BL] accumulators on DVE/ACT while x streams
in, reduced across partitions with 4 tiny ones-matmuls; rsqrt/64 is computed
as exp(-0.5*ln(var+eps) - ln 64) (ACT Rsqrt is banned for accuracy). Gate PSUM
groups are evacuated to SBUF immediately so PSUM banks never gate the PE
stream. Simulated end-to-end rel err of the fp8 pipeline: 7.4e-3 (gate 2e-2).
"""

import numpy as np
import ml_dtypes

import concourse.bacc as bacc
import concourse.bass as bass
import concourse.mybir as mybir
import concourse.tile as tile
from concourse import bass_utils

BF16 = ml_dtypes.bfloat16
F8 = ml_dtypes.float8_e4m3  # TRN FP8_EXP4 (max finite 240)
NCORES = 8
B, F, H = 8192, 2048, 1024
BL = B // NCORES          # 1024 rows per core
KC = F // 128             # 16 contraction chunks
SC = KC // 2              # 8 DoubleRow super-chunks
NB = 2                    # batch sub-chunks per core
BW = BL // NB             # 512 (one PSUM bank of fp32)
NG_LSTM = 2 * 8 * 3       # dir x h-chunk x {i,f,g,o}->{i,g,o} weight groups
EPS = 1e-5
WS = 64.0                 # fp8 weight pre-scale

_CACHE = {}


def _build_graph(has_c2=False):
    """has_c2: general path with a nonzero ln_b (rank-1 close matmuls on PE).
    The fast path (ln_b == 0) applies the -mu*c1 correction as a fused DVE
    scalar_tensor_tensor against a GPSIMD partition-broadcast of -mu, so PSUM
    groups close right at the last K-chunk and PE never waits on LN stats."""
    dt = mybir.dt
    AF = mybir.ActivationFunctionType
    OP = mybir.AluOpType
    DR = mybir.MatmulPerfMode.DoubleRow

    nc = bacc.Bacc("TRN2", target_bir_lowering=False, debug=False)

    xt_d = nc.dram_tensor("xt", (128, KC, BL), dt.bfloat16, kind="ExternalInput")
    xq_d = nc.dram_tensor("xq", (128, KC, BL), dt.float8e4, kind="ExternalInput")
    wg_d = nc.dram_tensor("wgm", (16, 128, KC, 128), dt.float8e4, kind="ExternalInput")
    wge_d = nc.dram_tensor("wge", (16, 2, 128), dt.bfloat16, kind="ExternalInput")
    wl_d = nc.dram_tensor("wlm", (16, 128, 3, KC, 128), dt.float8e4, kind="ExternalInput")
    bg_d = nc.dram_tensor("bgv", (128, 16), dt.float32, kind="ExternalInput")
    c1_d = nc.dram_tensor("c1v", (128, 16), dt.float32, kind="ExternalInput")
    bl_d = nc.dram_tensor("blv", (128, 48), dt.float32, kind="ExternalInput")
    w1_d = nc.dram_tensor("w1v", (128, KC * 64), dt.bfloat16, kind="ExternalInput")
    w2_d = nc.dram_tensor("w2v", (64, 1), dt.bfloat16, kind="ExternalInput")
    b1_d = nc.dram_tensor("b1v", (64, 1), dt.float32, kind="ExternalInput")
    b2_d = nc.dram_tensor("b2v", (1, 1), dt.float32, kind="ExternalInput")
    out_d = nc.dram_tensor("out", (1, BL), dt.float32, kind="ExternalOutput")

    with tile.TileContext(nc) as tc:
        with (
            tc.tile_pool(name="pers", bufs=1) as pers,
            tc.tile_pool(name="wpool", bufs=6) as wpool,
            tc.tile_pool(name="lpool", bufs=3) as lpool,
            tc.tile_pool(name="wepool", bufs=3) as wepool,
            tc.tile_pool(name="xsqp", bufs=3) as xsqp,
            tc.tile_pool(name="tmp", bufs=2) as tmp,
            tc.tile_pool(name="psum", bufs=8, space=bass.MemorySpace.PSUM) as psum,
        ):
            # ---- persistent SBUF tensors ----
            xsb = pers.tile([128, KC * BL], dt.bfloat16)
            xq3 = pers.tile([128, KC, BL], dt.float8e4)
            xg3 = pers.tile([128, KC, BL], dt.float8e4)
            feat = pers.tile([128, KC * BL], dt.bfloat16)
            hid = pers.tile([64, BL], dt.bfloat16)
            a_b = pers.tile([128, BL], dt.bfloat16)      # rsq/WS broadcast tile
            mu_b = pers.tile([128, BL], dt.bfloat16)     # -mu broadcast tile
            ones128 = pers.tile([128, 1], dt.bfloat16)
            # single-partition f32 rows (each costs free-bytes on every
            # partition, so reuse aggressively): s1row doubles as mu,
            # s2row as t1 -> ve -> lnv.
            s1row = pers.tile([1, BL], dt.float32)
            s2row = pers.tile([1, BL], dt.float32)
            rowtmp = pers.tile([1, BL], dt.float32)
            rsqb = pers.tile([1, BL], dt.bfloat16)
            orow = pers.tile([1, BL], dt.float32)
            xe = pers.tile([2, BL], dt.bfloat16)         # rank-1 rhs rows (-mu, sqrt(ve))
            xe1s = pers.tile([1, BL], dt.bfloat16)       # partition-0 staging for xe row 1
            bg_sb = pers.tile([128, 16], dt.float32)
            c1_sb = pers.tile([128, 16], dt.float32)
            bi_sb = pers.tile([128, 48], dt.float32)     # col = d*24 + part*8 + hm
            w1_sb = pers.tile([128, KC * 64], dt.bfloat16)
            w2_sb = pers.tile([64, 1], dt.bfloat16)
            b1_sb = pers.tile([64, 1], dt.float32)
            b2_sb = pers.tile([1, 1], dt.float32)

            nc.vector.memset(ones128[:], 1.0)

            # ---- input schedule. DMA queues fair-share HBM, so every
            # transfer in flight completes together. Emission order is the
            # priority order; everything after wave 1 is released by a single
            # dep on the last fp8-x chunk so wave 1 gets the full pipe:
            #   wave 1: gate j0/j1 weights (fp8, sub-split) + all 16 fp8 x
            #           chunks (~2.5MB) -> PE starts at ~7us
            #   wave 2 (fair-shared): gate j2..j15 weights (3.5MB), bf16 x
            #           (4MB, feeds LN stats whose PE reduction is emitted at
    #           j7 ~36us), consts; all land by ~28us
            #   wave 3: LSTM weights (12MB, consumed from ~72us; their wpool
            #           slots only free as the gate j-loop retires anyway)
            # NOTE: never chain DMAs across streams (an xt<-wg edge deadlocks:
            # wg slots free only after gate MMs that sit behind the stats MMs
            # on the in-order PE queue, and stats need xt).
            from concourse.tile_rust import add_dep_helper
            wgt01 = [wpool.tile([128, KC, 128], dt.float8e4, tag="w", name=f"wg{j}")
                     for j in range(2)]

            nc.sync.dma_start(wgt01[0][:], wg_d[0])
            xqdma = [nc.sync.dma_start(xq3[:, 0:8, :], xq_d[:, 0:8, :])]
            nc.sync.dma_start(wgt01[1][:], wg_d[1])
            xqdma.append(nc.sync.dma_start(xq3[:, 8:KC, :], xq_d[:, 8:KC, :]))

            # gate weights j2..15 right behind wave 1 (j2..5 can start once
            # the first xq half is in; the rest wait for all of wave 1)
            wgts = {0: wgt01[0], 1: wgt01[1]}
            for j in range(2, 16):
                wgt = wpool.tile([128, KC, 128], dt.float8e4, tag="w", name=f"wg{j}")
                wd = nc.sync.dma_start(wgt[:], wg_d[j])
                add_dep_helper(wd.ins, xqdma[0 if j < 6 else 1].ins,
                               reason="gate weight prefetch after fp8 x")
                wgts[j] = wgt

            # bf16 x stream fair-shares with the gate weights
            xdma = []
            for q in range(0, KC, 4):
                xd = nc.sync.dma_start(xsb[:, q * BL:(q + 4) * BL],
                                       xt_d[:, q:q + 4, :])
                add_dep_helper(xd.ins, xqdma[1].ins, reason="bf16 x after fp8 x")
                xdma.append(xd)

            # ---- constants / small DMAs (held behind the fp8 x stream) ----
            for sb_t, dr_t in [(bg_sb, bg_d), (c1_sb, c1_d), (bi_sb, bl_d),
                               (w1_sb, w1_d), (w2_sb, w2_d), (b1_sb, b1_d),
                               (b2_sb, b2_d)]:
                cd = nc.sync.dma_start(sb_t[:], dr_t[:, :])
                add_dep_helper(cd.ins, xqdma[1].ins, reason="const dma after x")

            # ---- LN statistics, off the PE hot path: fold the 16 K-chunks of
            # x (DVE) and x^2 (ACT squares + DVE folds) down to [128, BL]
            # accumulators while x streams in; the cross-partition reduction
            # is then just 4 tiny ones-matmuls emitted later in the gate loop.
            acc1 = pers.tile([128, BL], dt.bfloat16)
            acc2 = pers.tile([128, BL], dt.bfloat16)

            def emit_s2_folds():
                # all on DVE (squares as x*x muls, not ACT Square: the Scalar
                # engine is the LSTM-phase bottleneck and must enter it
                # drained), deferred past the early gate evacuations so PSUM
                # banks free the moment each group closes
                nc.vector.tensor_add(acc1[:], xsb[:, 0:BL], xsb[:, BL:2 * BL])
                for k in range(2, KC):
                    nc.vector.tensor_add(acc1[:], acc1[:], xsb[:, k * BL:(k + 1) * BL])
                sqs = []
                for k in range(2):
                    sq = xsqp.tile([128, BL], dt.bfloat16, tag="xsq", name=f"xsq{k}")
                    nc.vector.tensor_mul(sq[:], xsb[:, k * BL:(k + 1) * BL],
                                         xsb[:, k * BL:(k + 1) * BL])
                    sqs.append(sq)
                nc.vector.tensor_add(acc2[:], sqs[0][:], sqs[1][:])
                for k in range(2, KC):
                    sq = xsqp.tile([128, BL], dt.bfloat16, tag="xsq", name=f"xsq{k}")
                    nc.vector.tensor_mul(sq[:], xsb[:, k * BL:(k + 1) * BL],
                                         xsb[:, k * BL:(k + 1) * BL])
                    nc.vector.tensor_add(acc2[:], acc2[:], sq[:])

            def emit_stats_chain():
                # cross-partition reduce of the folded accumulators (4 tiny
                # matmuls), then the scalar LN chain and the -mu / rsq
                # partition-broadcasts. Nothing on the PE critical path
                # depends on this: gate PSUM groups are evacuated to SBUF
                # independently, and the first consumer (the gate epilogue
                # scalar_tensor_tensor) has plenty of slack.
                for b in range(NB):
                    sp1 = psum.tile([1, BW], dt.float32, tag="mm", name=f"s1p{b}")
                    nc.tensor.matmul(sp1[:], ones128[:], acc1[:, b * BW:(b + 1) * BW])
                    nc.vector.tensor_copy(s1row[:, b * BW:(b + 1) * BW], sp1[:])
                    sp2 = psum.tile([1, BW], dt.float32, tag="mm", name=f"s2p{b}")
                    nc.tensor.matmul(sp2[:], ones128[:], acc2[:, b * BW:(b + 1) * BW])
                    nc.vector.tensor_copy(s2row[:, b * BW:(b + 1) * BW], sp2[:])
                mu, lnv = s1row, s2row  # aliases after in-place updates below
                nc.vector.tensor_scalar_mul(mu[:], s1row[:], 1.0 / F)
                nc.vector.tensor_scalar(s2row[:], s2row[:], 1.0 / F, EPS, OP.mult, OP.add)
                nc.vector.tensor_mul(rowtmp[:], mu[:], mu[:])
                nc.vector.tensor_sub(s2row[:], s2row[:], rowtmp[:])      # = var+eps
                # pre-scale by WS^2 so exp(-0.5 ln(.)) comes out as rsq/WS,
                # undoing the fp8 weight pre-scale
                nc.vector.tensor_scalar_mul(s2row[:], s2row[:], WS * WS)
                nc.scalar.activation(lnv[:], s2row[:], AF.Ln)
                nc.scalar.activation(rsqb[:], lnv[:], AF.Exp, scale=-0.5)
                nc.vector.tensor_scalar_mul(xe[0:1, :], mu[:], -1.0)    # -> bf16
                # broadcast rsq and -mu to all partitions on the idle GPSIMD
                nc.gpsimd.partition_broadcast(a_b[:], rsqb[:])
                nc.gpsimd.partition_broadcast(mu_b[:], xe[0:1, :])
                if has_c2:
                    # engines can only write partition bases {0,32,64,96};
                    # stage row 1 on partition 0 and DMA it into place.
                    nc.scalar.activation(xe1s[:], lnv[:], AF.Exp, scale=0.5)
                    nc.sync.dma_start(xe[1:2, :], xe1s[:])

            def gate_epilogue(j, b, praw):
                # fast path: tpre = (-mu * c1_j + praw); general: praw complete
                if has_c2:
                    tps = tmp.tile([128, BW], dt.bfloat16, tag="tps", name=f"ts{j}_{b}")
                    nc.vector.tensor_mul(tps[:], praw[:], a_b[:, b * BW:(b + 1) * BW])
                else:
                    tpre = tmp.tile([128, BW], dt.bfloat16, tag="tpre", name=f"tp{j}_{b}")
                    nc.vector.scalar_tensor_tensor(
                        tpre[:], mu_b[:, b * BW:(b + 1) * BW], c1_sb[:, j:j + 1],
                        praw[:], OP.mult, OP.add,
                    )
                    tps = tmp.tile([128, BW], dt.bfloat16, tag="tps", name=f"ts{j}_{b}")
                    nc.vector.tensor_mul(tps[:], tpre[:], a_b[:, b * BW:(b + 1) * BW])
                gs = tmp.tile([128, BW], dt.bfloat16, tag="gs", name=f"gs{j}_{b}")
                nc.scalar.activation(gs[:], tps[:], AF.Sigmoid, bias=bg_sb[:, j:j + 1])
                # xg product on the otherwise-idle GPSIMD: keeps DVE free for
                # PSUM evacuations (its in-order queue recycles the banks)
                nc.gpsimd.tensor_mul(
                    xg3[:, j, b * BW:(b + 1) * BW],
                    xsb[:, j * BL + b * BW: j * BL + (b + 1) * BW], gs[:],
                )

            # ---- feature gate: 16 j-chunks x 8 DoubleRow super-chunks. PSUM
            # groups are evacuated to SBUF (bf16) right as they close; the
            # epilogues of j0..j3 are deferred until after the stats chain is
            # emitted so the DVE stream never blocks on the LN broadcasts.
            def gate_mm(gp, wgt, j, sc):
                out = []
                for b in range(NB):
                    out.append(nc.tensor.matmul(
                        gp[b][:], wgt[:, 2 * sc:2 * sc + 2, :],
                        xq3[:, 2 * sc:2 * sc + 2, b * BW:(b + 1) * BW],
                        start=(sc == 0), stop=(not has_c2 and sc == SC - 1),
                        perf_mode=DR,
                    ))
                return out

            def gate_close_evac(j, wet, gp, backlog):
                if has_c2:
                    for b in range(NB):
                        nc.tensor.matmul(
                            gp[b][:], wet[:], xe[:, b * BW:(b + 1) * BW],
                            start=False, stop=True,
                        )
                for b in range(NB):
                    praw = tmp.tile([128, BW], dt.bfloat16, tag="praw", bufs=20,
                                    name=f"pr{j}_{b}")
                    nc.vector.tensor_copy(praw[:], gp[b][:])
                    if backlog is not None:
                        backlog.append((j, b, praw))
                    else:
                        gate_epilogue(j, b, praw)

            backlog = []
            prev_last = None
            # Each j's first matmul is chained on the previous j's last: the
            # PE is serial anyway, but without the explicit edge the scheduler
            # interleaves the groups, closing them all late and starving PSUM
            # bank recycling (measured ~2-4us stall at j3/j4 on the bf16 ver).
            for j in range(16):
                wgt = wgts[j]
                wet = None
                if has_c2:
                    wet = wepool.tile([2, 128], dt.bfloat16, tag="we", name=f"we{j}")
                    nc.sync.dma_start(wet[:], wge_d[j, :, :])
                gp = [psum.tile([128, BW], dt.float32, tag="mm", name=f"gp{j}_{b}")
                      for b in range(NB)]
                for sc in range(SC):
                    mms = gate_mm(gp, wgt, j, sc)
                    if sc == 0 and prev_last is not None:
                        add_dep_helper(mms[0].ins, prev_last.ins,
                                       reason="gate groups close in order")
                prev_last = mms[-1]
                gate_close_evac(j, wet, gp, backlog if j <= 7 else None)
                if j == 4:
                    emit_s2_folds()
                elif j == 7:
                    # stats PE reduction sits between j7 and j8 on the in-order
                    # PE queue: by then bf16 x + folds are long done, and no
                    # DMA the stats depend on waits on a wpool slot freed by a
                    # post-stats matmul, so no cross-queue cycle.
                    emit_stats_chain()
                elif 8 <= j <= 11:
                    # flush the deferred epilogues four at a time so the j8+
                    # PSUM evacuations never queue behind the whole backlog on
                    # the in-order DVE stream
                    for (jj, bb, pr_) in backlog[:4]:
                        gate_epilogue(jj, bb, pr_)
                    backlog = backlog[4:]

            # ---- bidirectional 1-step LSTM (i, g, o only) ----
            for d in range(2):
                for hm in range(8):
                    dh = d * 8 + hm
                    # one DMA per (d,hm): all three i/g/o part blocks (0.75MB)
                    wlt = lpool.tile([128, 3, KC, 128], dt.float8e4, tag="wl",
                                     name=f"wl{dh}")
                    nc.sync.dma_start(wlt[:], wl_d[dh])
                    pp = []
                    for part in range(3):
                        g = dh * 3 + part
                        pb = [psum.tile([128, BW], dt.float32, tag="mm", name=f"lp{g}_{b}") for b in range(NB)]
                        for sc in range(SC):
                            for b in range(NB):
                                mm = nc.tensor.matmul(
                                    pb[b][:], wlt[:, part, 2 * sc:2 * sc + 2, :],
                                    xg3[:, 2 * sc:2 * sc + 2, b * BW:(b + 1) * BW],
                                    start=(sc == 0), stop=(sc == SC - 1),
                                    perf_mode=DR,
                                )
                                if sc == 0 and b == 0 and prev_last is not None:
                                    add_dep_helper(mm.ins, prev_last.ins,
                                                   reason="lstm groups close in order")
                        prev_last = mm
                        pp.append(pb)
                    for b in range(NB):
                        bcol = d * 24 + hm
                        ti = tmp.tile([128, BW], dt.bfloat16, tag="ti", name=f"ti{d}_{hm}_{b}")
                        nc.scalar.activation(ti[:], pp[0][b][:], AF.Sigmoid,
                                             bias=bi_sb[:, bcol:bcol + 1], scale=1.0 / WS)
                        tg = tmp.tile([128, BW], dt.bfloat16, tag="tg", name=f"tg{d}_{hm}_{b}")
                        nc.scalar.activation(tg[:], pp[1][b][:], AF.Tanh,
                                             bias=bi_sb[:, bcol + 8:bcol + 9], scale=1.0 / WS)
                        cb = tmp.tile([128, BW], dt.bfloat16, tag="cb", name=f"cb{d}_{hm}_{b}")
                        nc.vector.tensor_mul(cb[:], ti[:], tg[:])
                        tc2 = tmp.tile([128, BW], dt.bfloat16, tag="tc2", name=f"tc2{d}_{hm}_{b}")
                        nc.scalar.activation(tc2[:], cb[:], AF.Tanh)
                        to = tmp.tile([128, BW], dt.bfloat16, tag="to", name=f"to{d}_{hm}_{b}")
                        nc.scalar.activation(to[:], pp[2][b][:], AF.Sigmoid,
                                             bias=bi_sb[:, bcol + 16:bcol + 17], scale=1.0 / WS)
                        fc = d * 8 + hm
                        # final h product on GPSIMD (SBUF-only operands)
                        nc.gpsimd.tensor_mul(
                            feat[:, fc * BL + b * BW: fc * BL + (b + 1) * BW], to[:], tc2[:]
                        )

            # ---- head: relu(W1 . feat + b1), then W2 . hid + b2 ----
            for b in range(NB):
                hp = psum.tile([64, BW], dt.float32, tag="mm", name=f"hp{b}")
                for k in range(KC):
                    mm = nc.tensor.matmul(
                        hp[:], w1_sb[:, k * 64:(k + 1) * 64],
                        feat[:, k * BL + b * BW: k * BL + (b + 1) * BW],
                        start=(k == 0), stop=(k == KC - 1),
                    )
                    if k == 0:
                        add_dep_helper(mm.ins, prev_last.ins,
                                       reason="head groups close in order")
                prev_last = mm
                nc.scalar.activation(hid[:, b * BW:(b + 1) * BW], hp[:], AF.Relu, bias=b1_sb[:])
            for b in range(NB):
                op_ = psum.tile([1, BW], dt.float32, tag="mm", name=f"op{b}")
                nc.tensor.matmul(op_[:], w2_sb[:], hid[:, b * BW:(b + 1) * BW])
                nc.vector.tensor_scalar_add(orow[:, b * BW:(b + 1) * BW], op_[:], b2_sb[:])
                # ship each half as soon as its ADD lands (the single combined
                # DMA was observed firing ~5.7us after the data was ready)
                nc.sync.dma_start(out_d[:, b * BW:(b + 1) * BW], orow[:, b * BW:(b + 1) * BW])

    nc.compile()
    return nc


def _prep_inputs(x, ln_g, ln_b, Wg, bg, W_ih_f, b_ih_f, b_hh_f, W_ih_b, b_ih_b, b_hh_b,
                 W1, b1, W2, b2):
    """Host-side resharding/packing. All layouts are [partition, free]-grouped so
    every DMA lands as >=1KB contiguous runs per partition. Matmul weights are
    quantized to fp8-e4m3 after a x64 pre-scale."""
    f64 = np.float64

    def kgroup8(lhsT, mwidth):
        # lhsT [F, M] fp64 -> [M//mwidth groups][128 part][KC][mwidth] f8
        M = lhsT.shape[1]
        a = (lhsT * WS).astype(F8)
        a = a.reshape(KC, 128, M // mwidth, mwidth).transpose(2, 1, 0, 3)
        return np.ascontiguousarray(a)

    def kgroup(lhsT, mwidth):
        M = lhsT.shape[1]
        a = lhsT.reshape(KC, 128, M // mwidth, mwidth).transpose(2, 1, 0, 3)
        return np.ascontiguousarray(a.reshape(M // mwidth, 128, KC * mwidth)).astype(BF16)

    Wgl = (Wg.astype(f64) * ln_g.astype(f64)[None, :])
    wgm = kgroup8(np.ascontiguousarray(Wgl.T), 128)            # [16,128,16,128] f8
    # c1 from the quantized weights so the mu-correction matches the matmul
    Wq = wgm.astype(f64)                                       # [16,128,16,128]
    c1 = Wq.sum(axis=(1, 2)).reshape(16 * 128)                 # [2048] (x64 scale)
    # xe row 1 is exp(0.5*ln(WS^2*(var+eps))) = WS*sqrt(ve): c2 stays unscaled
    c2 = Wg.astype(f64) @ ln_b.astype(f64)                     # [2048]
    wge = np.stack([c1.reshape(16, 128), c2.reshape(16, 128)], axis=1).astype(BF16)

    idx = np.r_[0:H, 2 * H:3 * H, 3 * H:4 * H]                 # i, g, o rows
    wl_groups = []
    bl_all = np.zeros((128, 48), np.float32)
    for d, (Wih, bih, bhh) in enumerate(
        [(W_ih_f, b_ih_f, b_hh_f), (W_ih_b, b_ih_b, b_hh_b)]
    ):
        P = Wih[idx, :].astype(f64)                            # [3072, 2048]
        g24 = kgroup8(np.ascontiguousarray(P.T), 128)          # [24,128,16,128]
        for hm in range(8):
            # one [128, 3(part), KC, 128] block per (d,hm) -> single DMA
            wl_groups.append(np.stack([g24[part * 8 + hm] for part in range(3)], axis=1))
        bp = (bih.astype(f64) + bhh.astype(f64))[idx].astype(np.float32)
        bl_all[:, d * 24:(d + 1) * 24] = bp.reshape(24, 128).T  # col c = chunk p*8+hm
    wlm = np.ascontiguousarray(np.stack(wl_groups))            # [16,128,3,16,128]

    w1m = kgroup(np.ascontiguousarray(W1.T), 64)[0][None]      # [1,128,1024] -> squeeze
    w1m = np.ascontiguousarray(w1m[0])                         # [128, 16*64]
    w2m = np.ascontiguousarray(W2[0][:, None]).astype(BF16)    # [64,1]
    bgm = np.ascontiguousarray(bg.reshape(16, 128).T).astype(np.float32)  # [128,16]

    shared = {
        "wgm": wgm, "wge": wge, "wlm": wlm, "blv": bl_all, "bgv": bgm,
        "c1v": np.ascontiguousarray(c1.reshape(16, 128).T).astype(np.float32),
        "w1v": w1m, "w2v": w2m,
        "b1v": np.ascontiguousarray(b1[:, None]).astype(np.float32),
        "b2v": np.asarray(b2, np.float32).reshape(1, 1),
    }
    in_maps = []
    for c in range(NCORES):
        xs = x[c * BL:(c + 1) * BL, :].T                       # [2048, 1024]
        xt = np.ascontiguousarray(
            xs.reshape(KC, 128, BL).transpose(1, 0, 2)
        )                                                      # [128,16,1024] f32
        in_maps.append({"xt": xt.astype(BF16), "xq": xt.astype(F8), **shared})
    return in_maps


def _run(in_maps, trace=False, has_c2=False):
    key = ("nc", has_c2)
    if key not in _CACHE:
        _CACHE[key] = _build_graph(has_c2=has_c2)
    res = bass_utils.run_bass_kernel_spmd(
        _CACHE[key], in_maps, core_ids=list(range(NCORES)), trace=trace
    )
    return res


def kernel(x, ln_g, ln_b, Wg, bg,
           W_ih_f, W_hh_f, b_ih_f, b_hh_f,
           W_ih_b, W_hh_b, b_ih_b, b_hh_b,
           W1, b1, W2, b2, _trace=False, _return_res=False):
    args = [np.asarray(a) for a in (x, ln_g, ln_b, Wg, bg, W_ih_f, b_ih_f, b_hh_f,
                                    W_ih_b, b_ih_b, b_hh_b, W1, b1, W2, b2)]
    in_maps = _prep_inputs(*args)
    has_c2 = bool(np.any(np.asarray(ln_b) != 0))
    res = _run(in_maps, trace=_trace, has_c2=has_c2)
    out = np.concatenate(
        [np.asarray(res.results[c]["out"]).reshape(-1) for c in range(NCORES)]
    ).astype(np.float32)
    if _return_res:
        return out, res
    return out


# revision 17
# speedup vs baseline: 1.0717x; 1.0717x over previous
"""Trainium2 Bass kernel for nn_AFSLSTM (LayerNorm -> sigmoid feature gate ->
bidirectional 1-step LSTM -> tiny MLP head).

Sharding: data-parallel over the batch dim, 1024 rows per core, weights
replicated. No collectives; host concatenates per-core outputs.

Device-side math (per core, feature-major layout [feature_part, batch_free]):
the gate and LSTM matmuls (97% of PE work) run as fp8-e4m3 DoubleRow matmuls
(two 128-row contraction chunks per instruction, ~1.4-1.8x bf16 rate), with
weights pre-scaled by 64 on the host so the uniform(-1/32,1/32) entries sit
in fp8's normal range. The 1/64 descale folds into the LN rsqrt (gate) and
the activation `scale` operand (LSTM). x ships twice: bf16 for LN stats and
the xg product, fp8 as the gate moving operand; xg itself is written as fp8
by the DVE so the LSTM matmuls consume it directly. Head stays bf16.
  G0 = Q8(64*Wg*ln_g).Q8(x)               (8 DoubleRow matmuls per j-chunk)
  gate = sigmoid((rsq/64)*(G0 + (-mu)(x)c1q) + bg)   (c1q = colsums of the
        quantized weights, so the mu-correction matches the fp8 matmul)
  xg = fp8(x * gate)
  pre_{i,g,o} = Q8(64*W_ih[{i,g,o}]).xg    (f-gate dropped: c0 = 0)
  h = sigmoid(pre_o/64 + b_o) * tanh(sigmoid(pre_i/64 + b_i) * tanh(pre_g/64 + b_g))
  hid = relu(W1.feat + b1);  out = W2.hid + b2
LN statistics are folded to [128, BL] accumulators on DVE/ACT while x streams
in, reduced across partitions with 4 tiny ones-matmuls; rsqrt/64 is computed
as exp(-0.5*ln(var+eps) - ln 64) (ACT Rsqrt is banned for accuracy). Gate PSUM
groups are evacuated to SBUF immediately so PSUM banks never gate the PE
stream. Simulated end-to-end rel err of the fp8 pipeline: 7.4e-3 (gate 2e-2).
"""

import numpy as np
import ml_dtypes

import concourse.bacc as bacc
import concourse.bass as bass
import concourse.mybir as mybir
import concourse.tile as tile
from concourse import bass_utils

BF16 = ml_dtypes.bfloat16
F8 = ml_dtypes.float8_e4m3  # TRN FP8_EXP4 (max finite 240)
NCORES = 8
B, F, H = 8192, 2048, 1024
BL = B // NCORES          # 1024 rows per core
KC = F // 128             # 16 contraction chunks
SC = KC // 2              # 8 DoubleRow super-chunks
NB = 2                    # batch sub-chunks per core
BW = BL // NB             # 512 (one PSUM bank of fp32)
NG_LSTM = 2 * 8 * 3       # dir x h-chunk x {i,f,g,o}->{i,g,o} weight groups
EPS = 1e-5
WS = 64.0                 # fp8 weight pre-scale

_CACHE = {}


def _build_graph(has_c2=False):
    """has_c2: general path with a nonzero ln_b (rank-1 close matmuls on PE).
    The fast path (ln_b == 0) applies the -mu*c1 correction as a fused DVE
    scalar_tensor_tensor against a GPSIMD partition-broadcast of -mu, so PSUM
    groups close right at the last K-chunk and PE never waits on LN stats."""
    dt = mybir.dt
    AF = mybir.ActivationFunctionType
    OP = mybir.AluOpType
    DR = mybir.MatmulPerfMode.DoubleRow

    nc = bacc.Bacc("TRN2", target_bir_lowering=False, debug=False)

    xt_d = nc.dram_tensor("xt", (128, KC, BL), dt.bfloat16, kind="ExternalInput")
    xq_d = nc.dram_tensor("xq", (128, KC, BL), dt.float8e4, kind="ExternalInput")
    wg_d = nc.dram_tensor("wgm", (16, 128, KC, 128), dt.float8e4, kind="ExternalInput")
    wge_d = nc.dram_tensor("wge", (16, 2, 128), dt.bfloat16, kind="ExternalInput")
    wl_d = nc.dram_tensor("wlm", (16, 128, 3, KC, 128), dt.float8e4, kind="ExternalInput")
    bg_d = nc.dram_tensor("bgv", (128, 16), dt.float32, kind="ExternalInput")
    c1_d = nc.dram_tensor("c1v", (128, 16), dt.float32, kind="ExternalInput")
    bl_d = nc.dram_tensor("blv", (128, 48), dt.float32, kind="ExternalInput")
    w1_d = nc.dram_tensor("w1v", (128, KC * 64), dt.bfloat16, kind="ExternalInput")
    w2_d = nc.dram_tensor("w2v", (64, 1), dt.bfloat16, kind="ExternalInput")
    b1_d = nc.dram_tensor("b1v", (64, 1), dt.float32, kind="ExternalInput")
    b2_d = nc.dram_tensor("b2v", (1, 1), dt.float32, kind="ExternalInput")
    out_d = nc.dram_tensor("out", (1, BL), dt.float32, kind="ExternalOutput")

    with tile.TileContext(nc) as tc:
        with (
            tc.tile_pool(name="pers", bufs=1) as pers,
            tc.tile_pool(name="wpool", bufs=6) as wpool,
            tc.tile_pool(name="lpool", bufs=3) as lpool,
            tc.tile_pool(name="wepool", bufs=3) as wepool,
            tc.tile_pool(name="xsqp", bufs=3) as xsqp,
            tc.tile_pool(name="tmp", bufs=2) as tmp,
            tc.tile_pool(name="psum", bufs=8, space=bass.MemorySpace.PSUM) as psum,
        ):
            # ---- persistent SBUF tensors ----
            xsb = pers.tile([128, KC * BL], dt.bfloat16)
            xq3 = pers.tile([128, KC, BL], dt.float8e4)
            xg3 = pers.tile([128, KC, BL], dt.float8e4)
            feat = pers.tile([128, KC * BL], dt.bfloat16)
            hid = pers.tile([64, BL], dt.bfloat16)
            a_b = pers.tile([128, BL], dt.bfloat16)      # rsq/WS broadcast tile
            mu_b = pers.tile([128, BL], dt.bfloat16)     # -mu broadcast tile
            ones128 = pers.tile([128, 1], dt.bfloat16)
            # single-partition f32 rows (each costs free-bytes on every
            # partition, so reuse aggressively): s1row doubles as mu,
            # s2row as t1 -> ve -> lnv.
            s1row = pers.tile([1, BL], dt.float32)
            s2row = pers.tile([1, BL], dt.float32)
            rowtmp = pers.tile([1, BL], dt.float32)
            rsqb = pers.tile([1, BL], dt.bfloat16)
            orow = pers.tile([1, BL], dt.float32)
            xe = pers.tile([2, BL], dt.bfloat16)         # rank-1 rhs rows (-mu, sqrt(ve))
            xe1s = pers.tile([1, BL], dt.bfloat16)       # partition-0 staging for xe row 1
            bg_sb = pers.tile([128, 16], dt.float32)
            c1_sb = pers.tile([128, 16], dt.float32)
            bi_sb = pers.tile([128, 48], dt.float32)     # col = d*24 + part*8 + hm
            w1_sb = pers.tile([128, KC * 64], dt.bfloat16)
            w2_sb = pers.tile([64, 1], dt.bfloat16)
            b1_sb = pers.tile([64, 1], dt.float32)
            b2_sb = pers.tile([1, 1], dt.float32)

            nc.vector.memset(ones128[:], 1.0)

            # ---- input schedule. DMA queues fair-share HBM, so every
            # transfer in flight completes together. Emission order is the
            # priority order; everything after wave 1 is released by a single
            # dep on the last fp8-x chunk so wave 1 gets the full pipe:
            #   wave 1: gate j0/j1 weights (fp8, sub-split) + all 16 fp8 x
            #           chunks (~2.5MB) -> PE starts at ~7us
            #   wave 2 (fair-shared): gate j2..j15 weights (3.5MB), bf16 x
            #           (4MB, feeds LN stats whose PE reduction is emitted at
    #           j7 ~36us), consts; all land by ~28us
            #   wave 3: LSTM weights (12MB, consumed from ~72us; their wpool
            #           slots only free as the gate j-loop retires anyway)
            # NOTE: never chain DMAs across streams (an xt<-wg edge deadlocks:
            # wg slots free only after gate MMs that sit behind the stats MMs
            # on the in-order PE queue, and stats need xt).
            from concourse.tile_rust import add_dep_helper
            wgt01 = [wpool.tile([128, KC, 128], dt.float8e4, tag="w", name=f"wg{j}")
                     for j in range(2)]

            nc.sync.dma_start(wgt01[0][:], wg_d[0])
            xqdma = [nc.sync.dma_start(xq3[:, 0:8, :], xq_d[:, 0:8, :])]
            nc.sync.dma_start(wgt01[1][:], wg_d[1])
            xqdma.append(nc.sync.dma_start(xq3[:, 8:KC, :], xq_d[:, 8:KC, :]))

            # gate weights j2..15 right behind wave 1 (j2..5 can start once
            # the first xq half is in; the rest wait for all of wave 1)
            wgts = {0: wgt01[0], 1: wgt01[1]}
            for j in range(2, 16):
                wgt = wpool.tile([128, KC, 128], dt.float8e4, tag="w", name=f"wg{j}")
                wd = nc.sync.dma_start(wgt[:], wg_d[j])
                add_dep_helper(wd.ins, xqdma[0 if j < 6 else 1].ins,
                               reason="gate weight prefetch after fp8 x")
                wgts[j] = wgt

            # bf16 x stream fair-shares with the gate weights
            xdma = []
            for q in range(0, KC, 4):
                xd = nc.sync.dma_start(xsb[:, q * BL:(q + 4) * BL],
                                       xt_d[:, q:q + 4, :])
                add_dep_helper(xd.ins, xqdma[1].ins, reason="bf16 x after fp8 x")
                xdma.append(xd)

            # ---- constants / small DMAs (held behind the fp8 x stream) ----
            for sb_t, dr_t in [(bg_sb, bg_d), (c1_sb, c1_d), (bi_sb, bl_d),
                               (w1_sb, w1_d), (w2_sb, w2_d), (b1_sb, b1_d),
                               (b2_sb, b2_d)]:
                cd = nc.sync.dma_start(sb_t[:], dr_t[:, :])
                add_dep_helper(cd.ins, xqdma[1].ins, reason="const dma after x")

            # ---- LN statistics, off the PE hot path: fold the 16 K-chunks of
            # x (DVE) and x^2 (ACT squares + DVE folds) down to [128, BL]
            # accumulators while x streams in; the cross-partition reduction
            # is then just 4 tiny ones-matmuls emitted later in the gate loop.
            acc1 = pers.tile([128, BL], dt.bfloat16)
            acc2 = pers.tile([128, BL], dt.bfloat16)

            def emit_s2_folds():
                # all on DVE (squares as x*x muls, not ACT Square: the Scalar
                # engine is the LSTM-phase bottleneck and must enter it
                # drained), deferred past the early gate evacuations so PSUM
                # banks free the moment each group closes
                nc.vector.tensor_add(acc1[:], xsb[:, 0:BL], xsb[:, BL:2 * BL])
                for k in range(2, KC):
                    nc.vector.tensor_add(acc1[:], acc1[:], xsb[:, k * BL:(k + 1) * BL])
                sqs = []
                for k in range(2):
                    sq = xsqp.tile([128, BL], dt.bfloat16, tag="xsq", name=f"xsq{k}")
                    nc.vector.tensor_mul(sq[:], xsb[:, k * BL:(k + 1) * BL],
                                         xsb[:, k * BL:(k + 1) * BL])
                    sqs.append(sq)
                nc.vector.tensor_add(acc2[:], sqs[0][:], sqs[1][:])
                for k in range(2, KC):
                    sq = xsqp.tile([128, BL], dt.bfloat16, tag="xsq", name=f"xsq{k}")
                    nc.vector.tensor_mul(sq[:], xsb[:, k * BL:(k + 1) * BL],
                                         xsb[:, k * BL:(k + 1) * BL])
                    nc.vector.tensor_add(acc2[:], acc2[:], sq[:])

            def emit_stats_chain():
                # cross-partition reduce of the folded accumulators (4 tiny
                # matmuls), then the scalar LN chain and the -mu / rsq
                # partition-broadcasts. Nothing on the PE critical path
                # depends on this: gate PSUM groups are evacuated to SBUF
                # independently, and the first consumer (the gate epilogue
                # scalar_tensor_tensor) has plenty of slack.
                for b in range(NB):
                    sp1 = psum.tile([1, BW], dt.float32, tag="mm", name=f"s1p{b}")
                    nc.tensor.matmul(sp1[:], ones128[:], acc1[:, b * BW:(b + 1) * BW])
                    nc.vector.tensor_copy(s1row[:, b * BW:(b + 1) * BW], sp1[:])
                    sp2 = psum.tile([1, BW], dt.float32, tag="mm", name=f"s2p{b}")
                    nc.tensor.matmul(sp2[:], ones128[:], acc2[:, b * BW:(b + 1) * BW])
                    nc.vector.tensor_copy(s2row[:, b * BW:(b + 1) * BW], sp2[:])
                mu, lnv = s1row, s2row  # aliases after in-place updates below
                nc.vector.tensor_scalar_mul(mu[:], s1row[:], 1.0 / F)
                nc.vector.tensor_scalar(s2row[:], s2row[:], 1.0 / F, EPS, OP.mult, OP.add)
                nc.vector.tensor_mul(rowtmp[:], mu[:], mu[:])
                nc.vector.tensor_sub(s2row[:], s2row[:], rowtmp[:])      # = var+eps
                # pre-scale by WS^2 so exp(-0.5 ln(.)) comes out as rsq/WS,
                # undoing the fp8 weight pre-scale
                nc.vector.tensor_scalar_mul(s2row[:], s2row[:], WS * WS)
                nc.scalar.activation(lnv[:], s2row[:], AF.Ln)
                nc.scalar.activation(rsqb[:], lnv[:], AF.Exp, scale=-0.5)
                nc.vector.tensor_scalar_mul(xe[0:1, :], mu[:], -1.0)    # -> bf16
                # broadcast rsq and -mu to all partitions on the idle GPSIMD
                nc.gpsimd.partition_broadcast(a_b[:], rsqb[:])
                nc.gpsimd.partition_broadcast(mu_b[:], xe[0:1, :])
                if has_c2:
                    # engines can only write partition bases {0,32,64,96};
                    # stage row 1 on partition 0 and DMA it into place.
                    nc.scalar.activation(xe1s[:], lnv[:], AF.Exp, scale=0.5)
                    nc.sync.dma_start(xe[1:2, :], xe1s[:])

            def gate_epilogue(j, srcs):
                # srcs: per-bank praw sources (SBUF bf16 tiles for backlogged
                # groups, PSUM APs for live ones -- the stt doubles as the
                # evacuation, freeing the bank). tpre/tps/gs/xg run at full
                # [128, BL] width to halve DVE/ACT dispatch overhead.
                tpre = tmp.tile([128, BL], dt.bfloat16, tag="tpre", name=f"tp{j}")
                for b in range(NB):
                    if has_c2:
                        nc.vector.tensor_mul(tpre[:, b * BW:(b + 1) * BW], srcs[b][:],
                                             a_b[:, b * BW:(b + 1) * BW])
                    else:
                        nc.vector.scalar_tensor_tensor(
                            tpre[:, b * BW:(b + 1) * BW], mu_b[:, b * BW:(b + 1) * BW],
                            c1_sb[:, j:j + 1], srcs[b][:], OP.mult, OP.add,
                        )
                tps = tpre
                if not has_c2:
                    nc.vector.tensor_mul(tps[:], tpre[:], a_b[:])
                gs = tmp.tile([128, BL], dt.bfloat16, tag="gs", name=f"gs{j}")
                nc.scalar.activation(gs[:], tps[:], AF.Sigmoid, bias=bg_sb[:, j:j + 1])
                nc.vector.tensor_mul(xg3[:, j, :], xsb[:, j * BL:(j + 1) * BL], gs[:])

            # ---- feature gate: 16 j-chunks x 8 DoubleRow super-chunks. PSUM
            # groups are evacuated to SBUF (bf16) right as they close; the
            # epilogues of j0..j3 are deferred until after the stats chain is
            # emitted so the DVE stream never blocks on the LN broadcasts.
            def gate_mm(gp, wgt, j, sc):
                out = []
                for b in range(NB):
                    out.append(nc.tensor.matmul(
                        gp[b][:], wgt[:, 2 * sc:2 * sc + 2, :],
                        xq3[:, 2 * sc:2 * sc + 2, b * BW:(b + 1) * BW],
                        start=(sc == 0), stop=(not has_c2 and sc == SC - 1),
                        perf_mode=DR,
                    ))
                return out

            def gate_close_evac(j, wet, gp, backlog):
                if has_c2:
                    for b in range(NB):
                        nc.tensor.matmul(
                            gp[b][:], wet[:], xe[:, b * BW:(b + 1) * BW],
                            start=False, stop=True,
                        )
                if backlog is not None:
                    prs = []
                    for b in range(NB):
                        praw = tmp.tile([128, BW], dt.bfloat16, tag="praw", bufs=16,
                                        name=f"pr{j}_{b}")
                        # ScalarE copy: prompt bank release off the loaded DVE
                        nc.scalar.copy(praw[:], gp[b][:])
                        prs.append(praw)
                    backlog.append((j, prs))
                else:
                    gate_epilogue(j, gp)

            backlog = []
            prev_last = None
            # Each j's first matmul is chained on the previous j's last: the
            # PE is serial anyway, but without the explicit edge the scheduler
            # interleaves the groups, closing them all late and starving PSUM
            # bank recycling (measured ~2-4us stall at j3/j4 on the bf16 ver).
            for j in range(16):
                wgt = wgts[j]
                wet = None
                if has_c2:
                    wet = wepool.tile([2, 128], dt.bfloat16, tag="we", name=f"we{j}")
                    nc.sync.dma_start(wet[:], wge_d[j, :, :])
                gp = [psum.tile([128, BW], dt.float32, tag="mm", name=f"gp{j}_{b}")
                      for b in range(NB)]
                for sc in range(SC):
                    mms = gate_mm(gp, wgt, j, sc)
                    if sc == 0 and prev_last is not None:
                        add_dep_helper(mms[0].ins, prev_last.ins,
                                       reason="gate groups close in order")
                prev_last = mms[-1]
                gate_close_evac(j, wet, gp, backlog if j <= 7 else None)
                if j == 4:
                    emit_s2_folds()
                elif j == 7:
                    # stats PE reduction sits between j7 and j8 on the in-order
                    # PE queue: by then bf16 x + folds are long done, and no
                    # DMA the stats depend on waits on a wpool slot freed by a
                    # post-stats matmul, so no cross-queue cycle.
                    emit_stats_chain()
                elif 8 <= j <= 11:
                    # flush the deferred epilogues two groups at a time so the
                    # live-path stt (which is now also the PSUM evacuation)
                    # never queues behind the whole backlog on the DVE stream
                    for (jj, prs) in backlog[:2]:
                        gate_epilogue(jj, prs)
                    backlog = backlog[2:]

            # ---- bidirectional 1-step LSTM (i, g, o only) ----
            for d in range(2):
                for hm in range(8):
                    dh = d * 8 + hm
                    # one DMA per (d,hm): all three i/g/o part blocks (0.75MB)
                    wlt = lpool.tile([128, 3, KC, 128], dt.float8e4, tag="wl",
                                     name=f"wl{dh}")
                    nc.sync.dma_start(wlt[:], wl_d[dh])
                    pp = []
                    for part in range(3):
                        g = dh * 3 + part
                        pb = [psum.tile([128, BW], dt.float32, tag="mm", name=f"lp{g}_{b}") for b in range(NB)]
                        for sc in range(SC):
                            for b in range(NB):
                                mm = nc.tensor.matmul(
                                    pb[b][:], wlt[:, part, 2 * sc:2 * sc + 2, :],
                                    xg3[:, 2 * sc:2 * sc + 2, b * BW:(b + 1) * BW],
                                    start=(sc == 0), stop=(sc == SC - 1),
                                    perf_mode=DR,
                                )
                                if sc == 0 and b == 0 and prev_last is not None:
                                    add_dep_helper(mm.ins, prev_last.ins,
                                                   reason="lstm groups close in order")
                        prev_last = mm
                        pp.append(pb)
                    bcol = d * 24 + hm
                    ti = tmp.tile([128, BL], dt.bfloat16, tag="ti", name=f"ti{dh}")
                    tg = tmp.tile([128, BL], dt.bfloat16, tag="tg", name=f"tg{dh}")
                    to = tmp.tile([128, BL], dt.bfloat16, tag="to", name=f"to{dh}")
                    for b in range(NB):
                        bs = slice(b * BW, (b + 1) * BW)
                        nc.scalar.activation(ti[:, bs], pp[0][b][:], AF.Sigmoid,
                                             bias=bi_sb[:, bcol:bcol + 1], scale=1.0 / WS)
                        nc.scalar.activation(tg[:, bs], pp[1][b][:], AF.Tanh,
                                             bias=bi_sb[:, bcol + 8:bcol + 9], scale=1.0 / WS)
                        nc.scalar.activation(to[:, bs], pp[2][b][:], AF.Sigmoid,
                                             bias=bi_sb[:, bcol + 16:bcol + 17], scale=1.0 / WS)
                    cb = ti  # in-place: c = sigmoid(i)*tanh(g) overwrites ti
                    nc.vector.tensor_mul(cb[:], ti[:], tg[:])
                    tc2 = tg  # in-place: tanh(c) overwrites tg
                    nc.scalar.activation(tc2[:], cb[:], AF.Tanh)
                    nc.vector.tensor_mul(feat[:, dh * BL:(dh + 1) * BL], to[:], tc2[:])

            # ---- head: relu(W1 . feat + b1), then W2 . hid + b2 ----
            for b in range(NB):
                hp = psum.tile([64, BW], dt.float32, tag="mm", name=f"hp{b}")
                for k in range(KC):
                    mm = nc.tensor.matmul(
                        hp[:], w1_sb[:, k * 64:(k + 1) * 64],
                        feat[:, k * BL + b * BW: k * BL + (b + 1) * BW],
                        start=(k == 0), stop=(k == KC - 1),
                    )
                    if k == 0:
                        add_dep_helper(mm.ins, prev_last.ins,
                                       reason="head groups close in order")
                prev_last = mm
                nc.scalar.activation(hid[:, b * BW:(b + 1) * BW], hp[:], AF.Relu, bias=b1_sb[:])
            for b in range(NB):
                op_ = psum.tile([1, BW], dt.float32, tag="mm", name=f"op{b}")
                nc.tensor.matmul(op_[:], w2_sb[:], hid[:, b * BW:(b + 1) * BW])
                nc.vector.tensor_scalar_add(orow[:, b * BW:(b + 1) * BW], op_[:], b2_sb[:])
                # ship each half as soon as its ADD lands (the single combined
                # DMA was observed firing ~5.7us after the data was ready)
                nc.sync.dma_start(out_d[:, b * BW:(b + 1) * BW], orow[:, b * BW:(b + 1) * BW])

    nc.compile()
    return nc


def _prep_inputs(x, ln_g, ln_b, Wg, bg, W_ih_f, b_ih_f, b_hh_f, W_ih_b, b_ih_b, b_hh_b,
                 W1, b1, W2, b2):
    """Host-side resharding/packing. All layouts are [partition, free]-grouped so
    every DMA lands as >=1KB contiguous runs per partition. Matmul weights are
    quantized to fp8-e4m3 after a x64 pre-scale."""
    f64 = np.float64

    def kgroup8(lhsT, mwidth):
        # lhsT [F, M] fp64 -> [M//mwidth groups][128 part][KC][mwidth] f8
        M = lhsT.shape[1]
        a = (lhsT * WS).astype(F8)
        a = a.reshape(KC, 128, M // mwidth, mwidth).transpose(2, 1, 0, 3)
        return np.ascontiguousarray(a)

    def kgroup(lhsT, mwidth):
        M = lhsT.shape[1]
        a = lhsT.reshape(KC, 128, M // mwidth, mwidth).transpose(2, 1, 0, 3)
        return np.ascontiguousarray(a.reshape(M // mwidth, 128, KC * mwidth)).astype(BF16)

    Wgl = (Wg.astype(f64) * ln_g.astype(f64)[None, :])
    wgm = kgroup8(np.ascontiguousarray(Wgl.T), 128)            # [16,128,16,128] f8
    # c1 from the quantized weights so the mu-correction matches the matmul
    Wq = wgm.astype(f64)                                       # [16,128,16,128]
    c1 = Wq.sum(axis=(1, 2)).reshape(16 * 128)                 # [2048] (x64 scale)
    # xe row 1 is exp(0.5*ln(WS^2*(var+eps))) = WS*sqrt(ve): c2 stays unscaled
    c2 = Wg.astype(f64) @ ln_b.astype(f64)                     # [2048]
    wge = np.stack([c1.reshape(16, 128), c2.reshape(16, 128)], axis=1).astype(BF16)

    idx = np.r_[0:H, 2 * H:3 * H, 3 * H:4 * H]                 # i, g, o rows
    wl_groups = []
    bl_all = np.zeros((128, 48), np.float32)
    for d, (Wih, bih, bhh) in enumerate(
        [(W_ih_f, b_ih_f, b_hh_f), (W_ih_b, b_ih_b, b_hh_b)]
    ):
        P = Wih[idx, :].astype(f64)                            # [3072, 2048]
        g24 = kgroup8(np.ascontiguousarray(P.T), 128)          # [24,128,16,128]
        for hm in range(8):
            # one [128, 3(part), KC, 128] block per (d,hm) -> single DMA
            wl_groups.append(np.stack([g24[part * 8 + hm] for part in range(3)], axis=1))
        bp = (bih.astype(f64) + bhh.astype(f64))[idx].astype(np.float32)
        bl_all[:, d * 24:(d + 1) * 24] = bp.reshape(24, 128).T  # col c = chunk p*8+hm
    wlm = np.ascontiguousarray(np.stack(wl_groups))            # [16,128,3,16,128]

    w1m = kgroup(np.ascontiguousarray(W1.T), 64)[0][None]      # [1,128,1024] -> squeeze
    w1m = np.ascontiguousarray(w1m[0])                         # [128, 16*64]
    w2m = np.ascontiguousarray(W2[0][:, None]).astype(BF16)    # [64,1]
    bgm = np.ascontiguousarray(bg.reshape(16, 128).T).astype(np.float32)  # [128,16]

    shared = {
        "wgm": wgm, "wge": wge, "wlm": wlm, "blv": bl_all, "bgv": bgm,
        "c1v": np.ascontiguousarray(c1.reshape(16, 128).T).astype(np.float32),
        "w1v": w1m, "w2v": w2m,
        "b1v": np.ascontiguousarray(b1[:, None]).astype(np.float32),
        "b2v": np.asarray(b2, np.float32).reshape(1, 1),
    }
    in_maps = []
    for c in range(NCORES):
        xs = x[c * BL:(c + 1) * BL, :].T                       # [2048, 1024]
        xt = np.ascontiguousarray(
            xs.reshape(KC, 128, BL).transpose(1, 0, 2)
        )                                                      # [128,16,1024] f32
        in_maps.append({"xt": xt.astype(BF16), "xq": xt.astype(F8), **shared})
    return in_maps


def _run(in_maps, trace=False, has_c2=False):
    key = ("nc", has_c2)
    if key not in _CACHE:
        _CACHE[key] = _build_graph(has_c2=has_c2)
    res = bass_utils.run_bass_kernel_spmd(
        _CACHE[key], in_maps, core_ids=list(range(NCORES)), trace=trace
    )
    return res


def kernel(x, ln_g, ln_b, Wg, bg,
           W_ih_f, W_hh_f, b_ih_f, b_hh_f,
           W_ih_b, W_hh_b, b_ih_b, b_hh_b,
           W1, b1, W2, b2, _trace=False, _return_res=False):
    args = [np.asarray(a) for a in (x, ln_g, ln_b, Wg, bg, W_ih_f, b_ih_f, b_hh_f,
                                    W_ih_b, b_ih_b, b_hh_b, W1, b1, W2, b2)]
    in_maps = _prep_inputs(*args)
    has_c2 = bool(np.any(np.asarray(ln_b) != 0))
    res = _run(in_maps, trace=_trace, has_c2=has_c2)
    out = np.concatenate(
        [np.asarray(res.results[c]["out"]).reshape(-1) for c in range(NCORES)]
    ).astype(np.float32)
    if _return_res:
        return out, res
    return out
